# revision 21
# baseline (speedup 1.0000x reference)
"""MoE layer (16 experts, top-4, silu-gated FFN + shared expert) on 8 trn2 cores.

Strategy (expert-parallel, host-side dispatch):
  - Host computes the router (softmax + top-4 + renormalize) in numpy —
    0.2% of total FLOPs — and gathers each expert's tokens into a padded
    [capacity] batch (classic MoE dispatch, done host-side instead of
    device all-to-all).
  - Each of the 8 cores holds 2 experts (weights resident in SBUF, bf16)
    and runs the dense silu-gated FFN over its experts' gathered tokens,
    scaling activations by the combine weights before the down-projection
    so partial outputs can be scatter-added on the host.
  - Experts are ranked by token count: the 8 largest go to slot 0
    (capacity C0), the 8 smallest to slot 1 (capacity C1 <= C0) — less
    padding than one uniform capacity.
  - The shared expert is data-parallel: core i handles tokens
    [i*256, (i+1)*256).
  - All activations/weights are bf16 (PE: 1 cycle/row vs 2 for fp32),
    accumulation in fp32 PSUM.

Device layout: activations kept transposed (feature on partitions, tokens
on the free dim) so both matmuls feed the PE without any on-device
transpose; combine weights arrive pre-broadcast as [128, C] rows. All DRAM
tensors are partition-major ([128, k*f]) and x/outputs chunk-major, so
every DMA moves multi-KB contiguous segments per partition (1KB segments
cap the HWDGE queue at ~220 GB/s vs ~420 for 8KB). A run of dummy matmuls
on memset data at kernel start keeps the PE busy through the initial load
wait so the HAM clock-gate is released before real matmuls begin. Token
chunks are equal halves (<=512) so no chunk is so short that LDWEIGHTS
dominates.
"""

import os
import numpy as np
import ml_dtypes

DIM = 1024
HID = 512
E = 16
TOPK = 4
NCORES = 8
EPC = E // NCORES  # experts per core
T = 2048
S = T // NCORES  # shared-expert tokens per core

BF16 = ml_dtypes.bfloat16
OUT_BF16 = os.environ.get("KERNEL_OUT_F32", "0") != "1"

DK = DIM // 128   # 8 contraction tiles for the up-projections
HK = HID // 128   # 4 contraction tiles for the down-projection

_CACHE = {}


def _chunks(total):
    if total <= 512:
        return [(0, total)]
    nch = -(-total // 512)
    base = -(-total // (nch * 16)) * 16
    out, n0 = [], 0
    while n0 < total:
        n = min(base, total - n0)
        out.append((n0, n))
        n0 += n
    return out


def _build(caps):
    """Build + schedule the SPMD Tile kernel; caps = per-slot capacities."""
    import concourse.tile as tile
    import concourse.mybir as mybir
    from concourse import bacc

    f32 = mybir.dt.float32
    bf16 = mybir.dt.bfloat16
    fout = bf16 if OUT_BF16 else f32

    nc = bacc.Bacc("TRN2", target_bir_lowering=False, debug=False,
                   num_devices=NCORES)

    # per-slot DRAM tensors, partition-major; x and outputs chunk-major
    xe_d, cb_d, w1_d, w3_d, w2_d, o_d = [], [], [], [], [], []
    for j, Cj in enumerate(caps):
        xe_d.append(nc.dram_tensor(f"xe{j}", [128, DK * Cj], bf16,
                                   kind="ExternalInput"))
        cb_d.append(nc.dram_tensor(f"cb{j}", [128, Cj], f32,
                                   kind="ExternalInput"))
        if j == 0:
            w1_d.append([nc.dram_tensor(f"w1{j}a", [128, DK * (HID // 2)],
                                        bf16, kind="ExternalInput"),
                         nc.dram_tensor(f"w1{j}b", [128, DK * (HID // 2)],
                                        bf16, kind="ExternalInput")])
        else:
            w1_d.append(nc.dram_tensor(f"w1{j}", [128, DK * HID], bf16,
                                       kind="ExternalInput"))
        w3_d.append(nc.dram_tensor(f"w3{j}", [128, DK * HID], bf16,
                                   kind="ExternalInput"))
        w2_d.append(nc.dram_tensor(f"w2{j}", [128, HK * DIM], bf16,
                                   kind="ExternalInput"))
        o_d.append(nc.dram_tensor(f"o{j}", [128, DK * Cj], fout,
                                  kind="ExternalOutput"))
    xs = nc.dram_tensor("xs", [128, DK * S], bf16, kind="ExternalInput")
    ws1 = nc.dram_tensor("ws1", [128, DK * HID], bf16, kind="ExternalInput")
    ws3 = nc.dram_tensor("ws3", [128, DK * HID], bf16, kind="ExternalInput")
    ws2 = nc.dram_tensor("ws2", [128, HK * DIM], bf16, kind="ExternalInput")
    outs = nc.dram_tensor("outs", [128, DK * S], fout, kind="ExternalOutput")

    def k3(ap, k):
        return ap.rearrange("p (k f) -> p k f", k=k)

    with tile.TileContext(nc) as tc:
        with (
            tc.tile_pool(name="wts", bufs=1) as wts,
            tc.tile_pool(name="acts", bufs=1) as actp,
            tc.tile_pool(name="work", bufs=2) as work,
            tc.tile_pool(name="ost", bufs=2) as ostp,
            tc.tile_pool(name="ph", bufs=2, space="PSUM") as ph,
            tc.tile_pool(name="po", bufs=2, space="PSUM") as po,
        ):
            jobs = []
            # PE pre-warm: dummy matmuls on memset data run while the first
            # loads are in flight, so HAM un-throttles before real work.
            warm = work.tile([128, 512], bf16, tag="warm", name="warm",
                             bufs=1)
            nc.gpsimd.memset(warm[:], 0.0)
            pwarm = po.tile([128, 512], f32, tag="o", name="pwarm")
            for _ in range(52):
                nc.tensor.matmul(pwarm[:, 0:128], warm[:, 0:128],
                                 warm[:, 0:128], start=True, stop=True)

            def mk(t):
                return lambda k, fsl: t[:, k, fsl]

            for j, Cj in enumerate(caps):
                w3_t = wts.tile([128, DK, HID], bf16, name=f"w3t_{j}")
                w2_t = wts.tile([128, HK, DIM], bf16, name=f"w2t_{j}")
                cb_t = actp.tile([128, Cj], f32, name=f"cbt_{j}")
                if j == 0:
                    w1h_t = [wts.tile([128, DK, HID // 2], bf16,
                                      name=f"w1t_{j}{h}") for h in range(2)]
                    nc.sync.dma_start(out=w1h_t[0][:],
                                      in_=k3(w1_d[j][0][:], DK))
                    w1f_j = (lambda ts: lambda k, hsl:
                             ts[0 if hsl.start < HID // 2 else 1][
                                 :, k, hsl.start % (HID // 2):
                                 (hsl.start % (HID // 2)) + 128])(w1h_t)
                else:
                    w1_t = wts.tile([128, DK, HID], bf16, name=f"w1t_{j}")
                    nc.sync.dma_start(out=w1_t[:], in_=k3(w1_d[j][:], DK))
                    w1f_j = mk(w1_t)
                xts = []
                for ci, (n0, n) in enumerate(_chunks(Cj)):
                    xt = actp.tile([128, DK, n], bf16, name=f"xet_{j}_{ci}")
                    nc.sync.dma_start(
                        out=xt[:],
                        in_=xe_d[j][:, DK * n0:DK * (n0 + n)].rearrange(
                            "p (k t) -> p k t", k=DK))
                    xts.append(xt)
                if j == 0:
                    nc.sync.dma_start(out=w1h_t[1][:],
                                      in_=k3(w1_d[j][1][:], DK))
                nc.sync.dma_start(out=w3_t[:], in_=k3(w3_d[j][:], DK))
                nc.sync.dma_start(out=cb_t[:], in_=cb_d[j][:])
                nc.sync.dma_start(out=w2_t[:], in_=k3(w2_d[j][:], HK))

                def mkx(xts_):
                    return lambda ci, n0, n, k: xts_[ci][:, k, 0:n]
                jobs.append((w1f_j, mk(w3_t), mk(w2_t), mkx(xts), cb_t,
                             o_d[j], Cj))

            w1_s = wts.tile([128, DK, HID], bf16, name="sw1")
            w3_s = wts.tile([128, DK, HID], bf16, name="sw3")
            w2_s = wts.tile([128, HK, DIM], bf16, name="sw2")
            x_s = actp.tile([128, DK, S], bf16, name="xst")
            nc.sync.dma_start(out=w1_s[:], in_=k3(ws1[:], DK))
            nc.sync.dma_start(out=x_s[:], in_=k3(xs[:], DK))
            nc.sync.dma_start(out=w3_s[:], in_=k3(ws3[:], DK))
            nc.sync.dma_start(out=w2_s[:], in_=k3(ws2[:], HK))
            jobs.append((mk(w1_s), mk(w3_s), mk(w2_s),
                         lambda ci, n0, n, k: x_s[:, k, n0:n0 + n], None,
                         outs, S))

            items = [(job, ci, n0, n) for job in jobs
                     for ci, (n0, n) in enumerate(_chunks(job[6]))]
            for it_idx, (job, ci, n0, n) in enumerate(items):
                    (w1f, w3f, w2f, xf_, cb_t, o_ap, ntok) = job
                    is_last = it_idx == len(items) - 1
                    csl = slice(n0, n0 + n)
                    act_t = []
                    p1s = []
                    for hm in range(HK):
                        hsl = slice(hm * 128, (hm + 1) * 128)
                        p1 = ph.tile([128, 512], f32, tag=f"h1_{hm}",
                                     name="p1", bufs=1)
                        for k in range(DK):
                            nc.tensor.matmul(p1[:, :n], w1f(k, hsl),
                                             xf_(ci, n0, n, k),
                                             start=(k == 0),
                                             stop=(k == DK - 1))
                        p1s.append(p1)
                    for hm in range(HK):
                        hsl = slice(hm * 128, (hm + 1) * 128)
                        p1 = p1s[hm]
                        p3 = ph.tile([128, 512], f32, tag="h3", name="p3")
                        for k in range(DK):
                            nc.tensor.matmul(p3[:, :n], w3f(k, hsl),
                                             xf_(ci, n0, n, k),
                                             start=(k == 0),
                                             stop=(k == DK - 1))
                        sil = work.tile([128, 512], bf16, tag="sil",
                                        name="sil")
                        nc.scalar.activation(sil[:, :n], p1[:, :n],
                                             mybir.ActivationFunctionType.Silu)
                        a = work.tile([128, 512], bf16, tag=f"act{hm}",
                                      name=f"act{hm}")
                        if cb_t is not None:
                            h3s = work.tile([128, 512], bf16, tag="h3s",
                                            name="h3s")
                            nc.vector.tensor_tensor(h3s[:, :n], p3[:, :n],
                                                    cb_t[:, csl],
                                                    mybir.AluOpType.mult)
                            nc.vector.tensor_tensor(a[:, :n], h3s[:, :n],
                                                    sil[:, :n],
                                                    mybir.AluOpType.mult)
                        else:
                            nc.vector.tensor_tensor(a[:, :n], p3[:, :n],
                                                    sil[:, :n],
                                                    mybir.AluOpType.mult)
                        act_t.append(a)
                    stage = ostp.tile([128, DK, 512], fout, tag="stage",
                                      name="stage")
                    for dm in range(DK):
                        dsl = slice(dm * 128, (dm + 1) * 128)
                        pout = po.tile([128, 512], f32, tag="o", name="pout")
                        for k in range(HK):
                            nc.tensor.matmul(pout[:, :n], w2f(k, dsl),
                                             act_t[k][:, :n],
                                             start=(k == 0),
                                             stop=(k == HK - 1))
                        nc.vector.tensor_copy(out=stage[:, dm, :n],
                                              in_=pout[:, :n])
                    o_chunk = o_ap[:, DK * n0:DK * (n0 + n)].rearrange(
                        "p (k t) -> p k t", k=DK)
                    if is_last:
                        for d0 in range(0, DK, 2):
                            nc.sync.dma_start(
                                out=o_chunk[:, d0:d0 + 2, :],
                                in_=stage[:, d0:d0 + 2, :n])
                    else:
                        nc.sync.dma_start(out=o_chunk, in_=stage[:, :, :n])

    nc.compile()
    return nc


def _get_nc(caps):
    key = tuple(caps)
    if key not in _CACHE:
        _CACHE[key] = _build(caps)
    return _CACHE[key]


def _pmajor(a, nk):
    """[(k p), f] -> [128, k, f] partition-major for DMA-friendly rows."""
    kp, f = a.shape
    return np.ascontiguousarray(a.reshape(nk, 128, f).transpose(1, 0, 2))


def _chunk_major(pm, Cj):
    """[128, DK, Cj] -> [128, DK*Cj] with chunk-major column blocks."""
    return np.concatenate(
        [pm[:, :, n0:n0 + n].reshape(128, DK * n)
         for (n0, n) in _chunks(Cj)], axis=1)


LAST_RESULTS = None  # BassKernelResults from the most recent run (for test.py)


def kernel(x, gate_w, w1, w3, w2, sw1, sw3, sw2):
    global LAST_RESULTS
    from concourse.bass_utils import run_bass_kernel_spmd

    x = np.asarray(x)
    xf = np.ascontiguousarray(x.reshape(-1, DIM).astype(np.float32))
    gate_w = np.asarray(gate_w, dtype=np.float32)

    # ---- router on host (softmax -> top-4 -> renormalize) ----
    logits = xf @ gate_w.T                      # [T, E]
    m = logits.max(axis=1, keepdims=True)
    p = np.exp(logits - m)
    probs = p / p.sum(axis=1, keepdims=True)
    idx4 = np.argpartition(-probs, TOPK, axis=1)[:, :TOPK]     # [T, 4]
    w4 = np.take_along_axis(probs, idx4, axis=1)
    w4 = w4 / w4.sum(axis=1, keepdims=True)

    rows = np.repeat(np.arange(xf.shape[0]), TOPK)
    cols = idx4.ravel()
    vals = w4.ravel()

    tok_of = [rows[cols == e] for e in range(E)]
    cw_of = [vals[cols == e].astype(np.float32) for e in range(E)]
    counts = np.array([len(t) for t in tok_of])

    # rank experts by count: slot 0 gets the 8 largest, slot 1 the rest
    order = np.argsort(-counts, kind="stable")
    slot_experts = [order[j * NCORES:(j + 1) * NCORES] for j in range(EPC)]
    caps = [int(max(512, -(-counts[se].max() // 16) * 16))
            for se in slot_experts]

    xf_bf = xf.astype(BF16)
    w1 = np.asarray(w1, dtype=np.float32)
    w3 = np.asarray(w3, dtype=np.float32)
    w2 = np.asarray(w2, dtype=np.float32)
    sw1T = _pmajor(np.asarray(sw1, np.float32).T.astype(BF16), DK)
    sw3T = _pmajor(np.asarray(sw3, np.float32).T.astype(BF16), DK)
    sw2T = _pmajor(np.asarray(sw2, np.float32).T.astype(BF16), HK)

    in_maps = []
    for c in range(NCORES):
        im = {
            "xs": _pmajor(xf_bf[c * S:(c + 1) * S].T, DK
                          ).reshape(128, DK * S),
            "ws1": sw1T.reshape(128, DK * HID),
            "ws3": sw3T.reshape(128, DK * HID),
            "ws2": sw2T.reshape(128, HK * DIM),
        }
        for j, Cj in enumerate(caps):
            e = int(slot_experts[j][c])
            cnt = counts[e]
            pm = np.zeros((128, DK, Cj), dtype=BF16)
            g = xf_bf[tok_of[e]].T                 # [(k p), cnt]
            pm[:, :, :cnt] = g.reshape(DK, 128, cnt).transpose(1, 0, 2)
            cb_np = np.zeros((128, Cj), dtype=np.float32)
            cb_np[:, :cnt] = cw_of[e][None, :]
            im[f"xe{j}"] = _chunk_major(pm, Cj)
            im[f"cb{j}"] = cb_np
            w1pm = _pmajor(w1[e].T.astype(BF16), DK)
            if j == 0:
                im["w10a"] = np.ascontiguousarray(
                    w1pm[:, :, :HID // 2]).reshape(128, DK * (HID // 2))
                im["w10b"] = np.ascontiguousarray(
                    w1pm[:, :, HID // 2:]).reshape(128, DK * (HID // 2))
            else:
                im[f"w1{j}"] = w1pm.reshape(128, DK * HID)
            im[f"w3{j}"] = _pmajor(w3[e].T.astype(BF16), DK
                                   ).reshape(128, DK * HID)
            im[f"w2{j}"] = _pmajor(w2[e].T.astype(BF16), HK
                                   ).reshape(128, HK * DIM)
        in_maps.append(im)

    nc = _get_nc(caps)
    trace = os.environ.get("KERNEL_TRACE", "0") == "1"
    try:
        res = run_bass_kernel_spmd(nc, in_maps, core_ids=list(range(NCORES)),
                                   trace=trace)
    except Exception:
        # transient NRT device errors happen; one retry is usually enough
        res = run_bass_kernel_spmd(nc, in_maps, core_ids=list(range(NCORES)),
                                   trace=trace)
    LAST_RESULTS = res

    def decode(arr, ntok):
        """chunk-major [128, DK*ntok] -> [ntok, DIM] (token-major)."""
        outT = np.empty((DIM, ntok), dtype=np.float32)
        for (n0, n) in _chunks(ntok):
            blk = arr[:, DK * n0:DK * (n0 + n)].astype(np.float32)
            outT[:, n0:n0 + n] = blk.reshape(128, DK, n).transpose(
                1, 0, 2).reshape(DIM, n)
        return outT.T

    out = np.zeros((T, DIM), dtype=np.float32)
    for c in range(NCORES):
        r = res.results[c]
        for j, Cj in enumerate(caps):
            e = int(slot_experts[j][c])
            cnt = counts[e]
            out[tok_of[e]] += decode(r[f"o{j}"], Cj)[:cnt]
        out[c * S:(c + 1) * S] += decode(r["outs"], S)
    return out.reshape(x.shape).astype(np.float32)


# revision 22
# speedup vs baseline: 1.0039x; 1.0039x over previous
"""MoE layer (16 experts, top-4, silu-gated FFN + shared expert) on 8 trn2 cores.

Strategy (expert-parallel, host-side dispatch):
  - Host computes the router (softmax + top-4 + renormalize) in numpy —
    0.2% of total FLOPs — and gathers each expert's tokens into a padded
    [capacity] batch (classic MoE dispatch, done host-side instead of
    device all-to-all).
  - Each of the 8 cores holds 2 experts (weights resident in SBUF, bf16)
    and runs the dense silu-gated FFN over its experts' gathered tokens,
    scaling activations by the combine weights before the down-projection
    so partial outputs can be scatter-added on the host.
  - Experts are ranked by token count: the 8 largest go to slot 0
    (capacity C0), the 8 smallest to slot 1 (capacity C1 <= C0) — less
    padding than one uniform capacity.
  - The shared expert is data-parallel: core i handles tokens
    [i*256, (i+1)*256).
  - All activations/weights are bf16 (PE: 1 cycle/row vs 2 for fp32),
    accumulation in fp32 PSUM.

Device layout: activations kept transposed (feature on partitions, tokens
on the free dim) so both matmuls feed the PE without any on-device
transpose; combine weights arrive pre-broadcast as [128, C] rows. All DRAM
tensors are partition-major ([128, k*f]) and x/outputs chunk-major, so
every DMA moves multi-KB contiguous segments per partition (1KB segments
cap the HWDGE queue at ~220 GB/s vs ~420 for 8KB). A run of dummy matmuls
on memset data at kernel start keeps the PE busy through the initial load
wait so the HAM clock-gate is released before real matmuls begin. Token
chunks are equal halves (<=512) so no chunk is so short that LDWEIGHTS
dominates.
"""

import os
import numpy as np
import ml_dtypes

DIM = 1024
HID = 512
E = 16
TOPK = 4
NCORES = 8
EPC = E // NCORES  # experts per core
T = 2048
S = T // NCORES  # shared-expert tokens per core

BF16 = ml_dtypes.bfloat16
OUT_BF16 = os.environ.get("KERNEL_OUT_F32", "0") != "1"

DK = DIM // 128   # 8 contraction tiles for the up-projections
HK = HID // 128   # 4 contraction tiles for the down-projection

_CACHE = {}


def _chunks(total):
    if total <= 512:
        return [(0, total)]
    nch = -(-total // 512)
    base = -(-total // (nch * 16)) * 16
    out, n0 = [], 0
    while n0 < total:
        n = min(base, total - n0)
        out.append((n0, n))
        n0 += n
    return out


def _build(caps):
    """Build + schedule the SPMD Tile kernel; caps = per-slot capacities."""
    import concourse.tile as tile
    import concourse.mybir as mybir
    from concourse import bacc

    f32 = mybir.dt.float32
    bf16 = mybir.dt.bfloat16
    fout = bf16 if OUT_BF16 else f32

    nc = bacc.Bacc("TRN2", target_bir_lowering=False, debug=False,
                   num_devices=NCORES)

    # per-slot DRAM tensors, partition-major; x and outputs chunk-major
    xe_d, cb_d, w1_d, w3_d, w2_d, o_d = [], [], [], [], [], []
    for j, Cj in enumerate(caps):
        xe_d.append(nc.dram_tensor(f"xe{j}", [128, DK * Cj], bf16,
                                   kind="ExternalInput"))
        cb_d.append(nc.dram_tensor(f"cb{j}", [128, Cj], f32,
                                   kind="ExternalInput"))
        if j == 0:
            w1_d.append([nc.dram_tensor(f"w1{j}a", [128, DK * (HID // 2)],
                                        bf16, kind="ExternalInput"),
                         nc.dram_tensor(f"w1{j}b", [128, DK * (HID // 2)],
                                        bf16, kind="ExternalInput")])
        else:
            w1_d.append(nc.dram_tensor(f"w1{j}", [128, DK * HID], bf16,
                                       kind="ExternalInput"))
        w3_d.append(nc.dram_tensor(f"w3{j}", [128, DK * HID], bf16,
                                   kind="ExternalInput"))
        w2_d.append(nc.dram_tensor(f"w2{j}", [128, HK * DIM], bf16,
                                   kind="ExternalInput"))
        o_d.append(nc.dram_tensor(f"o{j}", [128, DK * Cj], fout,
                                  kind="ExternalOutput"))
    xs = nc.dram_tensor("xs", [128, DK * S], bf16, kind="ExternalInput")
    ws1 = nc.dram_tensor("ws1", [128, DK * HID], bf16, kind="ExternalInput")
    ws3 = nc.dram_tensor("ws3", [128, DK * HID], bf16, kind="ExternalInput")
    ws2 = nc.dram_tensor("ws2", [128, HK * DIM], bf16, kind="ExternalInput")
    outs = nc.dram_tensor("outs", [128, DK * S], fout, kind="ExternalOutput")

    def k3(ap, k):
        return ap.rearrange("p (k f) -> p k f", k=k)

    with tile.TileContext(nc) as tc:
        with (
            tc.tile_pool(name="wts", bufs=1) as wts,
            tc.tile_pool(name="acts", bufs=1) as actp,
            tc.tile_pool(name="work", bufs=2) as work,
            tc.tile_pool(name="ost", bufs=2) as ostp,
            tc.tile_pool(name="ph", bufs=2, space="PSUM") as ph,
            tc.tile_pool(name="po", bufs=2, space="PSUM") as po,
        ):
            jobs = []
            # PE pre-warm: dummy matmuls on memset data run while the first
            # loads are in flight, so HAM un-throttles before real work.
            warm = work.tile([128, 512], bf16, tag="warm", name="warm",
                             bufs=1)
            nc.gpsimd.memset(warm[:], 0.0)
            pwarm = po.tile([128, 512], f32, tag="o", name="pwarm")
            for _ in range(52):
                nc.tensor.matmul(pwarm[:, 0:128], warm[:, 0:128],
                                 warm[:, 0:128], start=True, stop=True)

            def mk(t):
                return lambda k, fsl: t[:, k, fsl]

            for j, Cj in enumerate(caps):
                w3_t = wts.tile([128, DK, HID], bf16, name=f"w3t_{j}")
                w2_t = wts.tile([128, HK, DIM], bf16, name=f"w2t_{j}")
                cb_t = actp.tile([128, Cj], f32, name=f"cbt_{j}")
                if j == 0:
                    w1h_t = [wts.tile([128, DK, HID // 2], bf16,
                                      name=f"w1t_{j}{h}") for h in range(2)]
                    nc.sync.dma_start(out=w1h_t[0][:],
                                      in_=k3(w1_d[j][0][:], DK))
                    w1f_j = (lambda ts: lambda k, hsl:
                             ts[0 if hsl.start < HID // 2 else 1][
                                 :, k, hsl.start % (HID // 2):
                                 (hsl.start % (HID // 2)) + 128])(w1h_t)
                else:
                    w1_t = wts.tile([128, DK, HID], bf16, name=f"w1t_{j}")
                    nc.sync.dma_start(out=w1_t[:], in_=k3(w1_d[j][:], DK))
                    w1f_j = mk(w1_t)
                xts = []
                for ci, (n0, n) in enumerate(_chunks(Cj)):
                    xt = actp.tile([128, DK, n], bf16, name=f"xet_{j}_{ci}")
                    nc.sync.dma_start(
                        out=xt[:],
                        in_=xe_d[j][:, DK * n0:DK * (n0 + n)].rearrange(
                            "p (k t) -> p k t", k=DK))
                    xts.append(xt)
                if j == 0:
                    nc.sync.dma_start(out=w1h_t[1][:],
                                      in_=k3(w1_d[j][1][:], DK))
                nc.sync.dma_start(out=w3_t[:], in_=k3(w3_d[j][:], DK))
                nc.sync.dma_start(out=cb_t[:], in_=cb_d[j][:])
                nc.sync.dma_start(out=w2_t[:], in_=k3(w2_d[j][:], HK))

                def mkx(xts_):
                    return lambda ci, n0, n, k: xts_[ci][:, k, 0:n]
                jobs.append((w1f_j, mk(w3_t), mk(w2_t), mkx(xts), cb_t,
                             o_d[j], Cj))

            w1_s = wts.tile([128, DK, HID], bf16, name="sw1")
            w3_s = wts.tile([128, DK, HID], bf16, name="sw3")
            w2_s = wts.tile([128, HK, DIM], bf16, name="sw2")
            x_s = actp.tile([128, DK, S], bf16, name="xst")
            nc.sync.dma_start(out=w1_s[:], in_=k3(ws1[:], DK))
            nc.sync.dma_start(out=x_s[:], in_=k3(xs[:], DK))
            nc.sync.dma_start(out=w3_s[:], in_=k3(ws3[:], DK))
            nc.sync.dma_start(out=w2_s[:], in_=k3(ws2[:], HK))
            jobs.append((mk(w1_s), mk(w3_s), mk(w2_s),
                         lambda ci, n0, n, k: x_s[:, k, n0:n0 + n], None,
                         outs, S))

            items = [(job, ci, n0, n) for job in jobs
                     for ci, (n0, n) in enumerate(_chunks(job[6]))]
            for it_idx, (job, ci, n0, n) in enumerate(items):
                    (w1f, w3f, w2f, xf_, cb_t, o_ap, ntok) = job
                    is_last = it_idx == len(items) - 1
                    csl = slice(n0, n0 + n)
                    act_t = []
                    p1s = []
                    for hm in range(HK):
                        hsl = slice(hm * 128, (hm + 1) * 128)
                        p1 = ph.tile([128, 512], f32, tag=f"h1_{hm}",
                                     name="p1", bufs=1)
                        for k in range(DK):
                            nc.tensor.matmul(p1[:, :n], w1f(k, hsl),
                                             xf_(ci, n0, n, k),
                                             start=(k == 0),
                                             stop=(k == DK - 1))
                        p1s.append(p1)
                    for hm in range(HK):
                        hsl = slice(hm * 128, (hm + 1) * 128)
                        p1 = p1s[hm]
                        p3 = ph.tile([128, 512], f32, tag="h3", name="p3")
                        for k in range(DK):
                            nc.tensor.matmul(p3[:, :n], w3f(k, hsl),
                                             xf_(ci, n0, n, k),
                                             start=(k == 0),
                                             stop=(k == DK - 1))
                        sil = work.tile([128, 512], bf16, tag="sil",
                                        name="sil")
                        nc.scalar.activation(sil[:, :n], p1[:, :n],
                                             mybir.ActivationFunctionType.Silu)
                        a = work.tile([128, 512], bf16, tag=f"act{hm}",
                                      name=f"act{hm}")
                        if cb_t is not None:
                            h3s = work.tile([128, 512], bf16, tag="h3s",
                                            name="h3s")
                            nc.vector.tensor_tensor(h3s[:, :n], p3[:, :n],
                                                    cb_t[:, csl],
                                                    mybir.AluOpType.mult)
                            nc.vector.tensor_tensor(a[:, :n], h3s[:, :n],
                                                    sil[:, :n],
                                                    mybir.AluOpType.mult)
                        else:
                            nc.vector.tensor_tensor(a[:, :n], p3[:, :n],
                                                    sil[:, :n],
                                                    mybir.AluOpType.mult)
                        act_t.append(a)
                    stage = ostp.tile([128, DK, 512], fout, tag="stage",
                                      name="stage")
                    for dm in range(DK):
                        dsl = slice(dm * 128, (dm + 1) * 128)
                        pout = po.tile([128, 512], f32, tag="o", name="pout")
                        for k in range(HK):
                            nc.tensor.matmul(pout[:, :n], w2f(k, dsl),
                                             act_t[k][:, :n],
                                             start=(k == 0),
                                             stop=(k == HK - 1))
                        nc.vector.tensor_copy(out=stage[:, dm, :n],
                                              in_=pout[:, :n])
                    o_chunk = o_ap[:, DK * n0:DK * (n0 + n)].rearrange(
                        "p (k t) -> p k t", k=DK)
                    if is_last:
                        for d0 in range(0, DK, 2):
                            nc.scalar.dma_start(
                                out=o_chunk[:, d0:d0 + 2, :],
                                in_=stage[:, d0:d0 + 2, :n])
                    else:
                        nc.sync.dma_start(out=o_chunk, in_=stage[:, :, :n])

    nc.compile()
    return nc


def _get_nc(caps):
    key = tuple(caps)
    if key not in _CACHE:
        _CACHE[key] = _build(caps)
    return _CACHE[key]


def _pmajor(a, nk):
    """[(k p), f] -> [128, k, f] partition-major for DMA-friendly rows."""
    kp, f = a.shape
    return np.ascontiguousarray(a.reshape(nk, 128, f).transpose(1, 0, 2))


def _chunk_major(pm, Cj):
    """[128, DK, Cj] -> [128, DK*Cj] with chunk-major column blocks."""
    return np.concatenate(
        [pm[:, :, n0:n0 + n].reshape(128, DK * n)
         for (n0, n) in _chunks(Cj)], axis=1)


LAST_RESULTS = None  # BassKernelResults from the most recent run (for test.py)


def kernel(x, gate_w, w1, w3, w2, sw1, sw3, sw2):
    global LAST_RESULTS
    from concourse.bass_utils import run_bass_kernel_spmd

    x = np.asarray(x)
    xf = np.ascontiguousarray(x.reshape(-1, DIM).astype(np.float32))
    gate_w = np.asarray(gate_w, dtype=np.float32)

    # ---- router on host (softmax -> top-4 -> renormalize) ----
    logits = xf @ gate_w.T                      # [T, E]
    m = logits.max(axis=1, keepdims=True)
    p = np.exp(logits - m)
    probs = p / p.sum(axis=1, keepdims=True)
    idx4 = np.argpartition(-probs, TOPK, axis=1)[:, :TOPK]     # [T, 4]
    w4 = np.take_along_axis(probs, idx4, axis=1)
    w4 = w4 / w4.sum(axis=1, keepdims=True)

    rows = np.repeat(np.arange(xf.shape[0]), TOPK)
    cols = idx4.ravel()
    vals = w4.ravel()

    tok_of = [rows[cols == e] for e in range(E)]
    cw_of = [vals[cols == e].astype(np.float32) for e in range(E)]
    counts = np.array([len(t) for t in tok_of])

    # rank experts by count: slot 0 gets the 8 largest, slot 1 the rest
    order = np.argsort(-counts, kind="stable")
    slot_experts = [order[j * NCORES:(j + 1) * NCORES] for j in range(EPC)]
    caps = [int(max(512, -(-counts[se].max() // 16) * 16))
            for se in slot_experts]

    xf_bf = xf.astype(BF16)
    w1 = np.asarray(w1, dtype=np.float32)
    w3 = np.asarray(w3, dtype=np.float32)
    w2 = np.asarray(w2, dtype=np.float32)
    sw1T = _pmajor(np.asarray(sw1, np.float32).T.astype(BF16), DK)
    sw3T = _pmajor(np.asarray(sw3, np.float32).T.astype(BF16), DK)
    sw2T = _pmajor(np.asarray(sw2, np.float32).T.astype(BF16), HK)

    in_maps = []
    for c in range(NCORES):
        im = {
            "xs": _pmajor(xf_bf[c * S:(c + 1) * S].T, DK
                          ).reshape(128, DK * S),
            "ws1": sw1T.reshape(128, DK * HID),
            "ws3": sw3T.reshape(128, DK * HID),
            "ws2": sw2T.reshape(128, HK * DIM),
        }
        for j, Cj in enumerate(caps):
            e = int(slot_experts[j][c])
            cnt = counts[e]
            pm = np.zeros((128, DK, Cj), dtype=BF16)
            g = xf_bf[tok_of[e]].T                 # [(k p), cnt]
            pm[:, :, :cnt] = g.reshape(DK, 128, cnt).transpose(1, 0, 2)
            cb_np = np.zeros((128, Cj), dtype=np.float32)
            cb_np[:, :cnt] = cw_of[e][None, :]
            im[f"xe{j}"] = _chunk_major(pm, Cj)
            im[f"cb{j}"] = cb_np
            w1pm = _pmajor(w1[e].T.astype(BF16), DK)
            if j == 0:
                im["w10a"] = np.ascontiguousarray(
                    w1pm[:, :, :HID // 2]).reshape(128, DK * (HID // 2))
                im["w10b"] = np.ascontiguousarray(
                    w1pm[:, :, HID // 2:]).reshape(128, DK * (HID // 2))
            else:
                im[f"w1{j}"] = w1pm.reshape(128, DK * HID)
            im[f"w3{j}"] = _pmajor(w3[e].T.astype(BF16), DK
                                   ).reshape(128, DK * HID)
            im[f"w2{j}"] = _pmajor(w2[e].T.astype(BF16), HK
                                   ).reshape(128, HK * DIM)
        in_maps.append(im)

    nc = _get_nc(caps)
    trace = os.environ.get("KERNEL_TRACE", "0") == "1"
    try:
        res = run_bass_kernel_spmd(nc, in_maps, core_ids=list(range(NCORES)),
                                   trace=trace)
    except Exception:
        # transient NRT device errors happen; one retry is usually enough
        res = run_bass_kernel_spmd(nc, in_maps, core_ids=list(range(NCORES)),
                                   trace=trace)
    LAST_RESULTS = res

    def decode(arr, ntok):
        """chunk-major [128, DK*ntok] -> [ntok, DIM] (token-major)."""
        outT = np.empty((DIM, ntok), dtype=np.float32)
        for (n0, n) in _chunks(ntok):
            blk = arr[:, DK * n0:DK * (n0 + n)].astype(np.float32)
            outT[:, n0:n0 + n] = blk.reshape(128, DK, n).transpose(
                1, 0, 2).reshape(DIM, n)
        return outT.T

    out = np.zeros((T, DIM), dtype=np.float32)
    for c in range(NCORES):
        r = res.results[c]
        for j, Cj in enumerate(caps):
            e = int(slot_experts[j][c])
            cnt = counts[e]
            out[tok_of[e]] += decode(r[f"o{j}"], Cj)[:cnt]
        out[c * S:(c + 1) * S] += decode(r["outs"], S)
    return out.reshape(x.shape).astype(np.float32)
